# revision 5
# baseline (speedup 1.0000x reference)
"""Trainium2 Bass kernel for strided-mask dense attention (nn_Attention_89283780149533).

Reference computation (b=2, n=2048, c=1024, 16 heads, hd=64, fp32):
    qkv = x @ W_qkv ; split into per-head q, k, v
    dots = (q @ k^T) * c**-0.5 ; masked to -inf where (i >= j) & ((i-j) % 32 == 0)
    out = softmax(dots) @ v ; out @ W_out + b_out

Sharding over 8 NeuronCores: core = batch*4 + head_group; each core handles one
batch element and 4 of the 16 heads (2 head pairs).  Each core computes a
partial output projection y_partial = attn_out[:, heads] @ W_out[head_rows, :];
the host sums the 4 partials per batch and adds b_out.

Device-side design (per core):
  - x arrives pre-transposed from the host, twice: bf16 (V path) and fp8-e4m3
    (Q/K path).  No PE transposes on device.
  - Q^T/K^T are produced by fp8 DoubleRow matmuls (K folded 2x256) against
    host-prescaled (x32) fp8 W_qk -> PSUM f32 -> bf16 SBUF.  S = K^T Q runs in
    bf16 with logits descaled inside the exp activation (scale = 1/32768).
  - The strided mask (i>=j & (i-j)%32==0) is rank-32 periodic for tiles fully
    below the diagonal: it is folded INTO the S matmul as 32 extra contraction
    rows (A[r,j]=1_{j%32==r} under K^T, B[r,i]=-393216*1_{i%32==r} under Q^T,
    i.e. -12 post-scale) at zero PE cost.  Only the 4 diagonal-band tiles per
    i-block need a 0/1 mask multiply on DVE after exp.
  - exp runs on the Activation engine ([128,1024] tiles, bf16 out).  attn@v
    uses V augmented with a ones column so row sums accumulate in PSUM row 64.
  - softmax normalization: DVE reciprocal of the rowsum row, gpsimd
    partition_broadcast (Pool engine) to spread it, DVE multiply into OT.
  - Output projection in bf16; PSUM->SBUF copies for y on the Pool engine.
"""

import sys
import numpy as np

if "/opt/trn_rl_repo" not in sys.path:
    sys.path.insert(0, "/opt/trn_rl_repo")

N_CORES = 8
B, N, C = 2, 2048, 1024
SIGMA = 1.0 / 32768.0  # post-matmul logit scale: c**-0.5 / (32*32 W prescale)
MASKB_VAL = -393216.0  # == -12.0 / SIGMA; exp(-12+s) ~ 1e-5 -> negligible

_CACHE = {}


def build_program(n_iters=1, interleave=False):
    from concourse import bacc
    import concourse.tile as tile
    import concourse.mybir as mybir

    f32 = mybir.dt.float32
    fr = mybir.dt.float32r
    bf = mybir.dt.bfloat16
    f8 = mybir.dt.float8e4
    DR = mybir.MatmulPerfMode.DoubleRow
    Exp = mybir.ActivationFunctionType.Exp

    nc = bacc.Bacc("TRN2", target_bir_lowering=False, debug=False,
                   num_devices=N_CORES)
    xbf = nc.dram_tensor("xbf", [C, N], bf, kind="ExternalInput").ap()
    x8 = nc.dram_tensor("x8", [C, N], f8, kind="ExternalInput").ap()
    wqk8 = nc.dram_tensor("wqk8", [C, 512], f8, kind="ExternalInput").ap()
    wv = nc.dram_tensor("wv", [C, 256], bf, kind="ExternalInput").ap()
    wo = nc.dram_tensor("wo", [256, C], bf, kind="ExternalInput").ap()
    mka = nc.dram_tensor("mka", [32, N], bf, kind="ExternalInput").ap()
    mkb = nc.dram_tensor("mkb", [32, N], bf, kind="ExternalInput").ap()
    mkd = nc.dram_tensor("mkd", [4, 128, 1024], bf, kind="ExternalInput").ap()
    onesv = nc.dram_tensor("onesv", [128, 64], f32, kind="ExternalInput").ap()
    y = nc.dram_tensor("y", [N, C], f32, kind="ExternalOutput").ap()

    with tile.TileContext(nc) as tc:
        with (
            tc.tile_pool(name="const", bufs=1) as const,
            tc.tile_pool(name="xin", bufs=2) as xin,
            tc.tile_pool(name="ptp", bufs=6) as pt_pool,
            tc.tile_pool(name="small", bufs=3) as small,
            tc.tile_pool(name="psA", bufs=2, space="PSUM") as psA,  # 2x[128,1024]
            tc.tile_pool(name="psB", bufs=4, space="PSUM") as psB,  # 4x[*,<=512]
        ):
            def body():
                wqk8_sb = const.tile([128, 8, 512], f8, tag="wqk8", name="wqk8")
                nc.sync.dma_start(wqk8_sb[:],
                                  wqk8.rearrange("(o p) m -> p o m", p=128))
                wv_sb = const.tile([128, 8, 256], bf, tag="wv", name="wv")
                nc.sync.dma_start(wv_sb[:],
                                  wv.rearrange("(o p) m -> p o m", p=128))
                wo_sb = const.tile([128, 2, 1024], bf, tag="wo", name="wo")
                nc.sync.dma_start(wo_sb[:],
                                  wo.rearrange("(o p) e -> p o e", p=128))
                mkd_sb = const.tile([128, 4, 1024], bf, tag="mkd", name="mkd")
                nc.sync.dma_start(mkd_sb[:], mkd.rearrange("m p f -> p m f"))
                onesv_sb = const.tile([128, 64], f32, tag="onesv", name="onesv")
                nc.sync.dma_start(onesv_sb[:], onesv)

                # Q^T/K^T with 32 mask rows appended (rows 64:96); per-head on
                # the free dim.  rows 0:64 filled by phase A copies.
                QTa_sb = const.tile([96, 4, N], bf, tag="qta", name="qta")
                KTa_sb = const.tile([96, 4, N], bf, tag="kta", name="kta")
                for h in range(4):
                    nc.sync.dma_start(QTa_sb[64:96, h, :], mkb)
                    nc.sync.dma_start(KTa_sb[64:96, h, :], mka)

                V_sb = const.tile([128, 4, 16, 65], bf, tag="v", name="v")
                OT_sb = const.tile([128, 2, N], bf, tag="ot", name="ot")
                nc.scalar.copy(V_sb[:, :, :, 64],
                               onesv_sb[:].rearrange("p (a b) -> p a b", a=4))

                ones1_sb = const.tile([1, 64], fr, tag="ones1", name="ones1")
                nc.scalar.copy(ones1_sb[:], onesv_sb[0:1, :])

                # ---- phase A building blocks: stream x^T; QK via fp8
                # DoubleRow; V in bf16.  Emitted as a list of PE work-groups
                # so they can be woven between attention steps.
                def chunk_dma(sc):
                    s0 = sc * 512
                    xbf_t = xin.tile([128, 8, 512], bf, tag="xbf", name="xbf")
                    nc.sync.dma_start(xbf_t[:],
                                      xbf.rearrange("(o p) n -> p o n",
                                                    p=128)[:, :, s0:s0 + 512])
                    x8_t = xin.tile([128, 8, 512], f8, tag="x8", name="x8")
                    nc.sync.dma_start(x8_t[:],
                                      x8.rearrange("(o p) n -> p o n",
                                                   p=128)[:, :, s0:s0 + 512])
                    return xbf_t, x8_t

                def gen_groups(sc, xbf_t, x8_t):
                    s0 = sc * 512

                    def qk_group(m):
                        def run():
                            ps = psB.tile([64, 512], f32, tag="ps", name="ps")
                            for c4 in range(4):
                                nc.tensor.matmul(
                                    ps[:],
                                    wqk8_sb[:, 2 * c4:2 * c4 + 2,
                                            m * 64:(m + 1) * 64],
                                    x8_t[:, 2 * c4:2 * c4 + 2, :],
                                    start=(c4 == 0), stop=(c4 == 3),
                                    perf_mode=DR)
                            dst = QTa_sb if m < 4 else KTa_sb
                            nc.vector.tensor_copy(
                                dst[0:64, m % 4, s0:s0 + 512], ps[:])
                        return run

                    def v_group(kb4):
                        def run():
                            ps = psB.tile([128, 256], f32, tag="ps", name="ps")
                            for cs in range(8):
                                nc.tensor.matmul(
                                    ps[:],
                                    xbf_t[:, cs, kb4 * 128:(kb4 + 1) * 128],
                                    wv_sb[:, cs, :],
                                    start=(cs == 0), stop=(cs == 7))
                            nc.vector.tensor_copy(
                                V_sb[:, :, sc * 4 + kb4, 0:64],
                                ps[:].rearrange("p (h d) -> p h d", h=4))
                        return run

                    return [qk_group(m) for m in range(8)] + \
                        [v_group(kb4) for kb4 in range(4)]

                def proj_groups(ib):
                    i0 = ib * 512

                    def proj_group(isub, cc):
                        def run():
                            py = psB.tile([128, 512], f32, tag="ps",
                                          name="py")
                            for go in range(2):
                                nc.tensor.matmul(
                                    py[:],
                                    OT_sb[:, go,
                                          i0 + isub * 128:i0 + (isub + 1) * 128],
                                    wo_sb[:, go, cc * 512:(cc + 1) * 512],
                                    start=(go == 0), stop=(go == 1))
                            ysb = small.tile([128, 512], f32, tag="ysb",
                                             name="ysb")
                            nc.vector.tensor_copy(ysb[:], py[:])
                            nc.sync.dma_start(
                                y[i0 + isub * 128:i0 + (isub + 1) * 128,
                                  cc * 512:(cc + 1) * 512], ysb[:])
                        return run

                    return [proj_group(isub, cc)
                            for isub in range(4) for cc in range(2)]

                # ---- phase B: attention; head pair shares a 2-bank S^T tile.
                # `weave` PE work-groups (QKV-gen / out-proj) are emitted
                # between attention steps to keep the PE dense while the
                # Activation engine grinds through the exps.
                xb0 = chunk_dma(0)
                groups0 = gen_groups(0, *xb0)
                for g in groups0:
                    g()

                def attention(ib, p, weave):
                    i0 = ib * 512
                    po = [psB.tile([65, 512], f32, tag="ps", name="po")
                          for _ in range(2)]

                    def S_step(jt):
                        # jt < 4*ib: fully-below-diag -> mask rows folded in
                        # (kp=96); 4*ib..4*ib+3: diagonal band -> DVE
                        # multiply; above: unmasked.
                        kp = 96 if jt < 4 * ib else 64
                        s2 = psA.tile([128, 1024], f32, tag="s2", name="s2")
                        for hh in range(2):
                            nc.tensor.matmul(
                                s2[:, hh * 512:(hh + 1) * 512],
                                KTa_sb[0:kp, 2 * p + hh,
                                       jt * 128:(jt + 1) * 128],
                                QTa_sb[0:kp, 2 * p + hh, i0:i0 + 512],
                                start=True, stop=True)
                        pt = pt_pool.tile([128, 1024], bf, tag="pt",
                                          name="pt")
                        nc.scalar.activation(pt[:], s2[:], Exp, scale=SIGMA)
                        d = jt - 4 * ib
                        if 0 <= d <= 3:
                            nc.vector.tensor_mul(pt[:], pt[:],
                                                 mkd_sb[:, d, :])
                        return pt

                    def AV_step(jt, pt):
                        for hh in range(2):
                            nc.tensor.matmul(
                                po[hh][:],
                                V_sb[:, 2 * p + hh, jt, :],
                                pt[:, hh * 512:(hh + 1) * 512],
                                start=(jt == 0), stop=(jt == 15))

                    buf = {}
                    for jt in range(16):
                        buf[jt] = S_step(jt)
                        for g in weave.get(jt, ()):
                            g()
                        if jt >= 2:
                            AV_step(jt - 2, buf.pop(jt - 2))
                    for jt in (14, 15):
                        AV_step(jt, buf.pop(jt))

                    # softmax normalization: row 64 of po holds sum_j P;
                    # 1/rowsum broadcast via a K=1 matmul into a PSUM carrier.
                    carrier = psA.tile([128, 1024], f32, tag="s2",
                                       name="carrier")
                    for hh in range(2):
                        rs = small.tile([1, 512], fr, tag="rs", name="rs")
                        with nc.allow_low_precision(
                                reason="f32r is full-width storage"):
                            nc.vector.reciprocal(rs[:], po[hh][64:65, :])
                        pb = carrier[0:64, hh * 512:(hh + 1) * 512]
                        nc.tensor.matmul(pb, ones1_sb[:], rs[:],
                                         start=True, stop=True)
                        pbs = small.tile([64, 512], fr, tag="pbs", name="pbs")
                        nc.vector.tensor_copy(pbs[:], pb)
                        if hh == 0:
                            nc.vector.tensor_mul(
                                OT_sb[0:64, p, i0:i0 + 512],
                                po[hh][0:64, :], pbs[:])
                        else:
                            tmp = small.tile([64, 512], bf, tag="tmp",
                                             name="tmp")
                            nc.vector.tensor_mul(tmp[:], po[hh][0:64, :],
                                                 pbs[:])
                            nc.sync.dma_start(
                                OT_sb[64:128, p, i0:i0 + 512], tmp[:])

                # ib0/p0: weave the remaining x chunks' QKV-gen between
                # attention steps (chunk sc complete before S reads it).
                weave0 = {}
                for sc in range(1, 4):
                    xb = chunk_dma(sc)
                    gs = gen_groups(sc, *xb)
                    for k in range(4):
                        weave0[(sc - 1) * 4 + k] = gs[3 * k:3 * k + 3]
                attention(0, 0, weave0)
                attention(0, 1, {})

                for ib in range(1, 4):
                    pg = proj_groups(ib - 1)
                    attention(ib, 0, {2 * k: [pg[k]] for k in range(4)})
                    attention(ib, 1, {2 * k: [pg[4 + k]] for k in range(4)})
                for g in proj_groups(3):
                    g()

            if n_iters > 1:
                with tc.For_i(0, n_iters, 1):
                    body()
            else:
                body()

    nc.compile()
    return nc


class Runner:
    """Cached jitted shard_map executor over the 8 axon cores (mirrors
    concourse.bass2jax.run_bass_via_pjrt but reusable across calls)."""

    def __init__(self, nc, n_cores=N_CORES):
        import jax
        from jax.sharding import Mesh, PartitionSpec, NamedSharding
        from jax.experimental.shard_map import shard_map
        import concourse.mybir as mybir
        from concourse import bass2jax
        from concourse.bass2jax import _bass_exec_p, install_neuronx_cc_hook

        install_neuronx_cc_hook()
        self.jax = jax
        self.nc = nc
        self.n_cores = n_cores
        partition_name = (nc.partition_id_tensor.name
                          if nc.partition_id_tensor else None)
        in_names, out_names, out_avals, zero_outs = [], [], [], []
        in_dtypes = {}
        for alloc in nc.m.functions[0].allocations:
            if not isinstance(alloc, mybir.MemoryLocationSet):
                continue
            name = alloc.memorylocations[0].name
            if alloc.kind == "ExternalInput":
                if name != partition_name:
                    in_names.append(name)
                    self_dt = mybir.dt.np(alloc.dtype)
                    in_dtypes[name] = self_dt
            elif alloc.kind == "ExternalOutput":
                out_names.append(name)
                shape = tuple(alloc.tensor_shape)
                dtype = mybir.dt.np(alloc.dtype)
                out_avals.append(jax.core.ShapedArray(shape, dtype))
                zero_outs.append(np.zeros(shape, dtype))
        self.in_names, self.out_names = in_names, out_names
        self.in_dtypes = in_dtypes
        self.out_avals, self.zero_outs = out_avals, zero_outs
        self.n_params = len(in_names)
        all_in_names = in_names + out_names
        if partition_name is not None:
            all_in_names.append(partition_name)

        def _body(*args):
            operands = list(args)
            if partition_name is not None:
                operands.append(bass2jax.partition_id_tensor())
            outs = _bass_exec_p.bind(
                *operands,
                out_avals=tuple(out_avals),
                in_names=tuple(all_in_names),
                out_names=tuple(out_names),
                lowering_input_output_aliases=(),
                sim_require_finite=True,
                sim_require_nnan=True,
                nc=nc,
            )
            return tuple(outs)

        devices = jax.devices()[:n_cores]
        self.mesh = Mesh(np.asarray(devices), ("core",))
        self.sharding = NamedSharding(self.mesh, PartitionSpec("core"))
        n_outs = len(out_names)
        in_specs = (PartitionSpec("core"),) * (self.n_params + n_outs)
        out_specs = (PartitionSpec("core"),) * n_outs
        self.fn = jax.jit(
            shard_map(_body, mesh=self.mesh, in_specs=in_specs,
                      out_specs=out_specs, check_rep=False),
            keep_unused=True,
        )

    def pack(self, in_maps):
        per_core = [[np.asarray(m[name]).astype(self.in_dtypes[name], copy=False)
                     for name in self.in_names]
                    for m in in_maps]
        concat_in = [
            np.concatenate([per_core[c][i] for c in range(self.n_cores)], axis=0)
            for i in range(self.n_params)
        ]
        concat_zeros = [
            np.zeros((self.n_cores * z.shape[0], *z.shape[1:]), z.dtype)
            for z in self.zero_outs
        ]
        return concat_in + concat_zeros

    def run(self, args):
        return self.fn(*args)

    def unpack(self, out_arrs):
        return [
            {name: np.asarray(out_arrs[i]).reshape(
                self.n_cores, *self.out_avals[i].shape)[c]
             for i, name in enumerate(self.out_names)}
            for c in range(self.n_cores)
        ]


def get_runner(n_iters=1, **kw):
    key = (n_iters, tuple(sorted(kw.items())))
    if key not in _CACHE:
        _CACHE[key] = Runner(build_program(n_iters=n_iters, **kw))
    return _CACHE[key]


def _mask_consts():
    import ml_dtypes
    r = np.arange(32)[:, None]
    j = np.arange(N)[None, :]
    mka = (j % 32 == r).astype(np.float32)          # [32, N] A pattern
    mkb = MASKB_VAL * (j % 32 == r)                 # [32, N] B pattern
    jj = np.arange(128)[:, None]
    ii = np.arange(512)[None, :]
    mkd = np.ones((4, 128, 512), np.float32)
    for d in range(4):
        mkd[d][((ii - jj - 128 * d) % 32 == 0) & (ii - jj >= 128 * d)] = 0.0
    mkd = np.concatenate([mkd, mkd], axis=2)        # [4, 128, 1024]
    bf = ml_dtypes.bfloat16
    return mka.astype(bf), mkb.astype(bf), mkd.astype(bf)


def shard_inputs(x, W_qkv, W_out):
    """Per-core input dicts: core = batch*4 + head_group."""
    import ml_dtypes
    bf = ml_dtypes.bfloat16
    f8 = ml_dtypes.float8_e4m3
    mka, mkb, mkd = _mask_consts()
    onesv = np.ones((128, 64), np.float32)
    in_maps = []
    xT = {}
    for bc in range(B):
        t = np.ascontiguousarray(x[bc].T)
        xT[bc] = (t.astype(bf), t.astype(f8))
    for core in range(N_CORES):
        bc, g = core // 4, core % 4
        cs = slice(g * 256, (g + 1) * 256)
        wqk = np.concatenate(
            [W_qkv[:, g * 256:(g + 1) * 256],
             W_qkv[:, 1024 + g * 256:1024 + (g + 1) * 256]], axis=1) * 32.0
        wv = W_qkv[:, 2048 + g * 256:2048 + (g + 1) * 256]
        in_maps.append({
            "xbf": xT[bc][0],
            "x8": xT[bc][1],
            "wqk8": wqk.astype(f8),
            "wv": np.ascontiguousarray(wv).astype(bf),
            "wo": np.ascontiguousarray(W_out[cs, :]).astype(bf),
            "mka": mka,
            "mkb": mkb,
            "mkd": mkd,
            "onesv": onesv,
        })
    return in_maps


def gather_output(results, b_out):
    y = np.empty((B, N, C), np.float32)
    for bc in range(B):
        acc = results[bc * 4]["y"].astype(np.float32).copy()
        for g in range(1, 4):
            acc += results[bc * 4 + g]["y"]
        y[bc] = acc
    return y + np.asarray(b_out, np.float32)[None, None, :]


def kernel(x, W_qkv, W_out, b_out):
    runner = get_runner()
    in_maps = shard_inputs(np.asarray(x, np.float32),
                           np.asarray(W_qkv, np.float32),
                           np.asarray(W_out, np.float32))
    args = runner.pack(in_maps)
    out = runner.run(args)
    self_jax = runner.jax
    self_jax.block_until_ready(out)
    results = runner.unpack(out)
    return gather_output(results, b_out)


if __name__ == "__main__":
    rng = np.random.default_rng(0)
    x = rng.standard_normal((B, N, C), dtype=np.float32)
    W_qkv = rng.standard_normal((C, 3 * C), dtype=np.float32) * 0.02
    W_out = rng.standard_normal((C, C), dtype=np.float32) * 0.02
    b_out = np.zeros((C,), np.float32)
    y = kernel(x, W_qkv, W_out, b_out)
    print("kernel output", y.shape, y.dtype, np.abs(y).mean())
